# revision 41
# baseline (speedup 1.0000x reference)
"""Trainium2 Bass kernel for MultiHeadBiasedAttentionADALN (B=1, N=768, D=768,
H=12, DP=128, DC=512), sequence-parallel over 8 NeuronCores.

Strategy (per core, SPMD over disjoint inputs — no collectives):
  - each core owns a 96-row block of queries / pair_rep rows; K/V computed
    from the full sequence on every core (weights replicated).
  - pair_rep is host-cast to bf16 and DMA'd with the xbar transpose so it
    arrives channel-major [128, pairs]; pair LN is done algebraically after
    the 128->12 projection (raw/mu/E2 ride one matmul stream each).
  - all big matmuls run in bf16; LN statistics accumulate in f32 PSUM.
"""

import hashlib
import math
import sys

import numpy as np

sys.path.insert(0, "/opt/trn_rl_repo")

import ml_dtypes

import concourse.bass as bass
import concourse.tile as tile
from concourse import mybir
from concourse.bass_utils import run_bass_kernel_spmd

BF16 = ml_dtypes.bfloat16
F32 = np.float32

N = 768          # sequence
D = 768          # model dim
H = 12           # heads
DH = 64          # head dim
DP = 128         # pair dim
DC = 512         # cond dim
NCORES = 8
R = N // NCORES  # 96 rows per core
NPAIR = R * N    # 73728 pairs per core
NCHUNK = NPAIR // 512   # 144 chunks of 512 pairs
PACK = 3                # chunks per psum pack (32-row slots at 0/32/64)
NPACKS = NCHUNK // PACK  # 48
EPS = 1e-5

bf = mybir.dt.bfloat16
f32 = mybir.dt.float32
Alu = mybir.AluOpType
Act = mybir.ActivationFunctionType


# --------------------------------------------------------------------------
# device program
# --------------------------------------------------------------------------

def _txp(nc, out_ap, in_ap):
    nc.sync.dma_start(out=out_ap, in_=in_ap, transpose=True)


_NO_SPLIT = {"InstCall", "InstNoOp", "InstBr", "InstBrCond",
             "InstSemaphoreOp", "InstEvSem", "InstBassTileRelease"}


def _split_excess_dma_waits(nc):
    """Most TPB instruction structs carry only one sync-wait slot, but Tile
    can assign several (DMAHW-lane WAW + multi-engine RAW).  Hoist the waits
    of any multi-wait instruction onto inserted same-engine NoOps (one wait
    each) — semantically identical, since the engine sequencer processes
    waits in order before issuing the instruction either way."""
    for bbname, bbw in nc.bb_map.items():
        bb = bbw.bb
        il = bb.instructions
        out = []
        changed = False
        for inst in il:
            si = getattr(inst, "sync_info", None)
            tname = type(inst).__name__
            if (tname not in _NO_SPLIT
                    and si is not None and si.on_wait
                    and len(si.on_wait) > 1):
                for w in si.on_wait:
                    nop = mybir.InstNoOp(
                        name=nc.get_next_instruction_name(), ins=[], outs=[],
                        text_hint="wait_split", bass_nofuse=True)
                    nop.engine = inst.engine
                    nop.sync_info = mybir.SyncInfo(on_wait=[w], on_update=[])
                    nc.register_instruction(nop)
                    out.append(nop)
                inst.sync_info = mybir.SyncInfo(
                    on_wait=[], on_update=list(si.on_update))
                changed = True
            out.append(inst)
        if changed:
            bb.instructions = out


def _ln_rowmajor(nc, pool, src, p, F, eps_sb):
    """LN stats for row-major [p, F] (PSUM or SBUF) -> (mean [p,1], rstd [p,1])."""
    sm = pool.tile([128, 2], f32, tag="ln_sums")
    nc.vector.reduce_sum(out=sm[:p, 0:1], in_=src, axis=mybir.AxisListType.X)
    sq = pool.tile([128, F], bf, tag="ln_sq")
    nc.vector.tensor_mul(sq[:p], src, src)
    nc.vector.reduce_sum(out=sm[:p, 1:2], in_=sq[:p],
                         axis=mybir.AxisListType.X)
    mean = pool.tile([128, 1], f32, tag="ln_mean")
    nc.vector.tensor_scalar_mul(mean[:p], sm[:p, 0:1], 1.0 / F)
    mm = pool.tile([128, 1], f32, tag="ln_mm")
    nc.vector.tensor_mul(mm[:p], mean[:p], mean[:p])
    var = pool.tile([128, 1], f32, tag="ln_var")
    nc.vector.scalar_tensor_tensor(out=var[:p], in0=sm[:p, 1:2],
                                   scalar=1.0 / F, in1=mm[:p],
                                   op0=Alu.mult, op1=Alu.subtract)
    rstd = pool.tile([128, 1], f32, tag="ln_rstd")
    nc.scalar.activation(out=rstd[:p], in_=var[:p], func=Act.Sqrt,
                         bias=eps_sb[:p], scale=1.0)
    nc.vector.reciprocal(out=rstd[:p], in_=rstd[:p])
    return mean[:p], rstd[:p]


def _qkln(nc, pool, ps, p, eps_sb, out_bf, gam=None, bet=None):
    """Per-head LN over DH for row-major [p, 768] psum -> out_bf bf16 [p,768]."""
    psh = ps.rearrange("p (h f) -> p h f", h=H)
    sums = pool.tile([128, H], f32, tag="qk_sums")
    nc.vector.reduce_sum(out=sums[:p], in_=psh, axis=mybir.AxisListType.X)
    sq = pool.tile([128, D], bf, tag="qk_sq")
    nc.scalar.square(out=sq[:p], in_=ps)
    e2s = pool.tile([128, H], f32, tag="qk_e2s")
    nc.vector.reduce_sum(out=e2s[:p], in_=sq[:p].rearrange(
        "p (h f) -> p h f", h=H), axis=mybir.AxisListType.X)
    mean = pool.tile([128, H], f32, tag="qk_mean")
    nc.vector.tensor_scalar_mul(mean[:p], sums[:p], 1.0 / DH)
    mm = pool.tile([128, H], f32, tag="qk_mm")
    nc.vector.tensor_mul(mm[:p], mean[:p], mean[:p])
    var = pool.tile([128, H], f32, tag="qk_var")
    nc.vector.scalar_tensor_tensor(out=var[:p], in0=e2s[:p],
                                   scalar=1.0 / DH, in1=mm[:p],
                                   op0=Alu.mult, op1=Alu.subtract)
    rstd = pool.tile([128, H], f32, tag="qk_rstd")
    nc.scalar.activation(out=rstd[:p], in_=var[:p], func=Act.Sqrt,
                         bias=eps_sb[:p], scale=1.0)
    nc.vector.reciprocal(out=rstd[:p], in_=rstd[:p])
    outh = out_bf.rearrange("p (h f) -> p h f", h=H)
    for h in range(H):
        nc.vector.tensor_scalar(
            out=outh[:p, h, :], in0=psh[:, h, :],
            scalar1=mean[:p, h:h + 1], scalar2=rstd[:p, h:h + 1],
            op0=Alu.subtract, op1=Alu.mult)
    if gam is not None:
        nc.vector.tensor_mul(out_bf[:p], out_bf[:p], gam[:p])
    if bet is not None:
        nc.vector.tensor_add(out_bf[:p], out_bf[:p], bet[:p])


def build_program(flags):
    """flags: dict of structural switches (see kernel())."""
    nc = bass.Bass()

    # ---- dram I/O ----
    pair_bf = nc.dram_tensor("pair_bf", [NPAIR, DP], bf, kind="ExternalInput")
    x_all = nc.dram_tensor("x_all", [N, D], bf, kind="ExternalInput")
    cond_all = nc.dram_tensor("cond_all", [N, DC], bf, kind="ExternalInput")
    x_rows = nc.dram_tensor("x_rows", [R, D], bf, kind="ExternalInput")
    cond_rows = nc.dram_tensor("cond_rows", [R, DC], bf, kind="ExternalInput")
    Wg = nc.dram_tensor("Wg", [DC, D], bf, kind="ExternalInput")
    Wb = nc.dram_tensor("Wb", [DC, D], bf, kind="ExternalInput")
    Wsc = nc.dram_tensor("Wsc", [DC, D], bf, kind="ExternalInput")
    Wq = nc.dram_tensor("Wq", [D, D], bf, kind="ExternalInput")
    Wk = nc.dram_tensor("Wk", [D, D], bf, kind="ExternalInput")
    Wv = nc.dram_tensor("Wv", [D, D], bf, kind="ExternalInput")
    Wo = nc.dram_tensor("Wo", [D, D], bf, kind="ExternalInput")
    Wpb = nc.dram_tensor("Wpb", [DP, 64], bf, kind="ExternalInput")
    c0t = nc.dram_tensor("c0t", [R, H], f32, kind="ExternalInput")
    w1t = nc.dram_tensor("w1t", [R, H], f32, kind="ExternalInput")  # -w1
    out_t = nc.dram_tensor("out", [R, D], f32, kind="ExternalOutput")

    opt_in = {}
    def opt_tensor(name, shape, dt=f32):
        opt_in[name] = nc.dram_tensor(name, shape, dt, kind="ExternalInput")
        return opt_in[name]

    if flags["bg"]:
        bg_t = opt_tensor("bg_t", [128, 6])
    if flags["bq"]:
        bq_t = opt_tensor("bq_t", [128, D])
    if flags["bk"]:
        bk_t = opt_tensor("bk_t", [128, D])
    if flags["bv"]:
        bv_t = opt_tensor("bv_t", [128, D])
    if flags["bo"]:
        bo_t = opt_tensor("bo_t", [128, D])
    if flags["qg"]:
        qg_t = opt_tensor("qg_t", [128, D])
    if flags["qb"]:
        qb_t = opt_tensor("qb_t", [128, D])
    if flags["kg"]:
        kg_t = opt_tensor("kg_t", [128, D])
    if flags["kb"]:
        kb_t = opt_tensor("kb_t", [128, D])
    if flags["cg"]:
        cg_t = opt_tensor("cg_t", [128, DC])
    if flags["cb"]:
        cbeta_t = opt_tensor("cbeta_t", [128, DC])
    if flags["sbvec"]:
        sb_t = opt_tensor("sb_t", [128, D])
    if flags["mask"]:
        cbias_t = opt_tensor("cbias_t", [128, N])

    with tile.TileContext(nc) as tc:
        ctx_pools = []

        singles_cm = tc.tile_pool(name="singles", bufs=1)
        singles = singles_cm.__enter__(); ctx_pools.append(singles_cm)

        eps_sb = singles.tile([128, 1], f32)
        nc.vector.memset(eps_sb, EPS)
        zero_sb = singles.tile([128, 1], f32)
        nc.vector.memset(zero_sb, 0.0)

        # ---- load weights ----
        def load_w(dram, parts, free, name):
            ts = []
            for i in range(parts // 128):
                t = singles.tile([128, free], bf, tag=f"{name}{i}", name=f"{name}{i}")
                nc.sync.dma_start(out=t, in_=dram[i * 128:(i + 1) * 128, :])
                ts.append(t)
            return ts
        Wg_sb = load_w(Wg, DC, D, "Wg")
        Wb_sb = load_w(Wb, DC, D, "Wb")
        Wsc_sb = load_w(Wsc, DC, D, "Wsc")
        Wq_sb = load_w(Wq, D, D, "Wq")
        Wk_sb = load_w(Wk, D, D, "Wk")
        Wv_sb = load_w(Wv, D, D, "Wv")
        Wo_sb = load_w(Wo, D, D, "Wo")
        Wpb_sb = singles.tile([128, 64], bf)
        nc.sync.dma_start(out=Wpb_sb, in_=Wpb[:, :])
        c0_sb = singles.tile([R, H], f32)
        nc.sync.dma_start(out=c0_sb, in_=c0t[:, :])
        w1_sb = singles.tile([R, H], f32)
        nc.sync.dma_start(out=w1_sb, in_=w1t[:, :])

        def load_opt(name, shape, tag):
            t = singles.tile(shape, f32, tag=tag, name=tag)
            nc.sync.dma_start(out=t, in_=opt_in[name][:, :])
            return t
        bg_sb = load_opt("bg_t", [128, 6], "bg") if flags["bg"] else None
        bq_sb = load_opt("bq_t", [128, D], "bq") if flags["bq"] else None
        bk_sb = load_opt("bk_t", [128, D], "bk") if flags["bk"] else None
        bv_sb = load_opt("bv_t", [128, D], "bv") if flags["bv"] else None
        bo_sb = load_opt("bo_t", [128, D], "bo") if flags["bo"] else None
        qg_sb = load_opt("qg_t", [128, D], "qg") if flags["qg"] else None
        qb_sb = load_opt("qb_t", [128, D], "qb") if flags["qb"] else None
        kg_sb = load_opt("kg_t", [128, D], "kg") if flags["kg"] else None
        kb_sb = load_opt("kb_t", [128, D], "kb") if flags["kb"] else None
        cg_sb = load_opt("cg_t", [128, DC], "cg") if flags["cg"] else None
        cbeta_sb = load_opt("cbeta_t", [128, DC], "cbeta") if flags["cb"] else None
        sb_sb = load_opt("sb_t", [128, D], "sbv") if flags["sbvec"] else None
        cbias_sb = load_opt("cbias_t", [128, N], "cbias") if flags["mask"] else None

        # persistent activations
        hT = [singles.tile([128, N], bf, tag=f"hT{i}", name=f"hT{i}") for i in range(6)]
        hrT = [singles.tile([128, R], bf, tag=f"hrT{i}", name=f"hrT{i}") for i in range(6)]
        knT = [singles.tile([128, N], bf, tag=f"knT{i}", name=f"knT{i}") for i in range(6)]
        qnT = [singles.tile([128, R], bf, tag=f"qnT{i}", name=f"qnT{i}") for i in range(6)]
        v_sb = [singles.tile([128, D], bf, tag=f"v{i}", name=f"v{i}") for i in range(6)]
        crT = [singles.tile([128, R], bf, tag=f"crT{i}", name=f"crT{i}") for i in range(4)]

        dram_cm = tc.tile_pool(name="dramp", bufs=1, space="DRAM")
        dram_pool = dram_cm.__enter__(); ctx_pools.append(dram_cm)
        rmt = dram_pool.tile([14, NCHUNK, 512], bf)

        # ================= P1: adaln -> hT (full) and hrT (rows) ===========
        with tc.tile_pool(name="p1", bufs=3) as p1, \
             tc.tile_pool(name="p1keep", bufs=1) as p1keep, \
             tc.tile_pool(name="p1ps", bufs=2, space="PSUM") as p1ps:

            def adaln(x_dram, cond_dram, p, nrt, hT_out, tagp):
                # returns nothing; fills hT_out tiles [128, p]
                nblk = (p + 127) // 128
                xn_tiles = []
                cnT_tiles = [p1keep.tile([128, p], bf, tag=f"{tagp}cnT{c}",
                                         name=f"{tagp}cnT{c}") for c in range(4)]
                for ib in range(nblk):
                    pp = min(128, p - ib * 128)
                    xt = p1.tile([128, D], bf, tag=f"{tagp}x")
                    nc.sync.dma_start(out=xt[:pp], in_=x_dram[ib * 128:ib * 128 + pp, :])
                    m, r_ = _ln_rowmajor(nc, p1, xt[:pp], pp, D, eps_sb)
                    xn = p1keep.tile([128, D], bf, tag=f"{tagp}xn{ib}",
                                     name=f"{tagp}xn{ib}")
                    nc.vector.tensor_scalar(out=xn[:pp], in0=xt[:pp],
                                            scalar1=m, scalar2=r_,
                                            op0=Alu.subtract, op1=Alu.mult)
                    xn_tiles.append(xn)
                    ct = p1.tile([128, DC], bf, tag=f"{tagp}c")
                    nc.sync.dma_start(out=ct[:pp],
                                      in_=cond_dram[ib * 128:ib * 128 + pp, :])
                    m2, r2 = _ln_rowmajor(nc, p1, ct[:pp], pp, DC, eps_sb)
                    cn = p1.tile([128, DC], bf, tag=f"{tagp}cn")
                    nc.vector.tensor_scalar(out=cn[:pp], in0=ct[:pp],
                                            scalar1=m2, scalar2=r2,
                                            op0=Alu.subtract, op1=Alu.mult)
                    if flags["cg"]:
                        nc.vector.tensor_mul(cn[:pp], cn[:pp], cg_sb[:pp])
                    if flags["cb"]:
                        nc.vector.tensor_add(cn[:pp], cn[:pp], cbeta_sb[:pp])
                    for c in range(4):
                        _txp(nc, cnT_tiles[c][:, ib * 128:ib * 128 + pp],
                             cn[:pp, c * 128:(c + 1) * 128])
                # per feature-block: g/b args transposed, then hT
                for fb in range(6):
                    xnT = p1.tile([128, p], bf, tag=f"{tagp}xnT")
                    for ib in range(nblk):
                        pp = min(128, p - ib * 128)
                        _txp(nc, xnT[:, ib * 128:ib * 128 + pp],
                             xn_tiles[ib][:pp, fb * 128:(fb + 1) * 128])
                    gps = p1ps.tile([128, nrt], f32, tag="gps", name="gps")
                    bps = p1ps.tile([128, nrt], f32, tag="bps", name="bps")
                    nsplit = [(0, min(512, p))] + ([(512, p - 512)] if p > 512 else [])
                    for (o0, nn) in nsplit:
                        for c in range(4):
                            nc.tensor.matmul(
                                gps[:, o0:o0 + nn],
                                lhsT=Wg_sb[c][:, fb * 128:(fb + 1) * 128],
                                rhs=cnT_tiles[c][:, o0:o0 + nn],
                                start=(c == 0), stop=(c == 3))
                        for c in range(4):
                            nc.tensor.matmul(
                                bps[:, o0:o0 + nn],
                                lhsT=Wb_sb[c][:, fb * 128:(fb + 1) * 128],
                                rhs=cnT_tiles[c][:, o0:o0 + nn],
                                start=(c == 0), stop=(c == 3))
                    sig = p1.tile([128, p], bf, tag=f"{tagp}sig")
                    nc.scalar.activation(
                        out=sig[:, :p], in_=gps[:, :p], func=Act.Sigmoid,
                        bias=(bg_sb[:, fb:fb + 1] if flags["bg"] else zero_sb),
                        scale=1.0)
                    tmp = p1.tile([128, p], bf, tag=f"{tagp}tmp")
                    nc.vector.tensor_mul(tmp[:, :p], xnT[:, :p], sig[:, :p])
                    nc.vector.tensor_add(hT_out[fb][:, :p], tmp[:, :p], bps[:, :p])

            adaln(x_all, cond_all, N, N, hT, "f")
            adaln(x_rows, cond_rows, R, R, hrT, "r")

            # raw cond_rows transposed (for the output gate)
            craw = p1.tile([128, DC], bf, tag="craw")
            nc.sync.dma_start(out=craw[:R], in_=cond_rows[:, :])
            for c in range(4):
                _txp(nc, crT[c][:, :R], craw[:R, c * 128:(c + 1) * 128])

        # ================= P2: k/v (full rows), q (own rows) ===============
        with tc.tile_pool(name="p2", bufs=3) as p2, \
             tc.tile_pool(name="p2ps", bufs=2, space="PSUM") as p2ps:
            for ib in range(6):
                kps = p2ps.tile([128, D], f32, tag="kps")
                vps = p2ps.tile([128, D], f32, tag="vps")
                for (o0, nn) in [(0, 512), (512, 256)]:
                    for c in range(6):
                        nc.tensor.matmul(
                            kps[:, o0:o0 + nn],
                            lhsT=hT[c][:, ib * 128:(ib + 1) * 128],
                            rhs=Wk_sb[c][:, o0:o0 + nn],
                            start=(c == 0), stop=(c == 5))
                    for c in range(6):
                        nc.tensor.matmul(
                            vps[:, o0:o0 + nn],
                            lhsT=hT[c][:, ib * 128:(ib + 1) * 128],
                            rhs=Wv_sb[c][:, o0:o0 + nn],
                            start=(c == 0), stop=(c == 5))
                if flags["bk"]:
                    nc.vector.tensor_add(kps[:, :], kps[:, :], bk_sb[:, :])
                if flags["bv"]:
                    nc.vector.tensor_add(vps[:, :], vps[:, :], bv_sb[:, :])
                kn = p2.tile([128, D], bf, tag="kn")
                _qkln(nc, p2, kps, 128, eps_sb, kn,
                      gam=(kg_sb if flags["kg"] else None),
                      bet=(kb_sb if flags["kb"] else None))
                for fb in range(6):
                    _txp(nc, knT[fb][:, ib * 128:(ib + 1) * 128],
                         kn[:, fb * 128:(fb + 1) * 128])
                nc.scalar.copy(out=v_sb[ib][:, :], in_=vps[:, :])

            qps = p2ps.tile([128, D], f32, tag="kps")
            for (o0, nn) in [(0, 512), (512, 256)]:
                for c in range(6):
                    nc.tensor.matmul(
                        qps[:R, o0:o0 + nn],
                        lhsT=hrT[c][:, :R],
                        rhs=Wq_sb[c][:, o0:o0 + nn],
                        start=(c == 0), stop=(c == 5))
            if flags["bq"]:
                nc.vector.tensor_add(qps[:R, :], qps[:R, :], bq_sb[:R, :])
            qn = p2.tile([128, D], bf, tag="kn")
            _qkln(nc, p2, qps[:R], R, eps_sb, qn[:R],
                  gam=(qg_sb[:R] if flags["qg"] else None),
                  bet=(qb_sb[:R] if flags["qb"] else None))
            for fb in range(6):
                _txp(nc, qnT[fb][:, :R], qn[:R, fb * 128:(fb + 1) * 128])

        # ================= P3: pair stream -> rmt ==========================
        with tc.tile_pool(name="p3", bufs=2) as p3, \
             tc.tile_pool(name="p3ps", bufs=2, space="PSUM") as p3ps:
            for g in range(NPACKS):
                pbT = p3.tile([128, PACK * 512], bf, tag="pbT")
                _txp(nc, pbT,
                     pair_bf[g * PACK * 512:(g + 1) * PACK * 512, :])
                sq = p3.tile([128, PACK * 512], bf, tag="sq")
                nc.vector.tensor_mul(sq, pbT, pbT)
                pk = p3ps.tile([96, 512], f32, tag="pk")
                for j in range(PACK):
                    # rows 32j..32j+14: [raw(12) | mu | E2] (+18 zero rows)
                    # via two accumulated matmuls, 32-wide zero-padded
                    # stationary so the full 32-row slot is written
                    nc.tensor.matmul(pk[32 * j:32 * j + 32, :],
                                     lhsT=Wpb_sb[:, 0:32],
                                     rhs=pbT[:, j * 512:(j + 1) * 512],
                                     start=True, stop=False)
                    nc.tensor.matmul(pk[32 * j:32 * j + 32, :],
                                     lhsT=Wpb_sb[:, 32:64],
                                     rhs=sq[:, j * 512:(j + 1) * 512],
                                     start=False, stop=True)
                rm = p3.tile([96, 512], bf, tag="rm")
                nc.scalar.copy(out=rm, in_=pk)
                for j in range(PACK):
                    nc.sync.dma_start(out=rmt[:, g * PACK + j, :],
                                      in_=rm[32 * j:32 * j + 14, :])

        # ================= P4: attention ===================================
        with tc.tile_pool(name="p4", bufs=3) as p4, \
             tc.tile_pool(name="p4s", bufs=1) as p4s:

            # pair stats (once)
            mu = p4s.tile([R, N], bf)
            nc.sync.dma_start(out=mu, in_=rmt[12].rearrange("c f -> (c f)")
                              .rearrange("(p f) -> p f", p=R))
            e2 = p4s.tile([R, N], bf)
            nc.sync.dma_start(out=e2, in_=rmt[13].rearrange("c f -> (c f)")
                              .rearrange("(p f) -> p f", p=R))
            m2 = p4s.tile([R, N], f32)
            nc.vector.tensor_mul(m2, mu, mu)
            ve = p4s.tile([R, N], f32)
            nc.vector.scalar_tensor_tensor(out=ve, in0=e2, scalar=EPS,
                                           in1=m2, op0=Alu.add,
                                           op1=Alu.subtract)
            rr = p4s.tile([R, N], f32)
            nc.vector.reciprocal(out=rr, in_=ve)
            rp = p4s.tile([R, N], f32)
            nc.scalar.activation(out=rp, in_=rr, func=Act.Sqrt,
                                 bias=zero_sb[:R], scale=1.0)
            mr = p4s.tile([R, N], f32)
            nc.vector.tensor_mul(mr, mu, rp)

            den = p4s.tile([R, H], f32)
            p4po_cm = tc.tile_pool(name="p4po", bufs=1, space="PSUM")
            p4po = p4po_cm.__enter__()
            p4ps_cm = tc.tile_pool(name="p4ps", bufs=2, space="PSUM")
            p4ps = p4ps_cm.__enter__()
            ops = p4po.tile([R, D], f32)

            for h in range(H):
                raw = p4.tile([R, N], bf, tag="raw")
                nc.sync.dma_start(out=raw, in_=rmt[h]
                                  .rearrange("c f -> (c f)")
                                  .rearrange("(p f) -> p f", p=R))
                qk = p4ps.tile([R, N], f32, tag="qk")
                t6 = h // 2
                lo = 64 * (h % 2)
                for (o0, nn) in [(0, 512), (512, 256)]:
                    nc.tensor.matmul(qk[:, o0:o0 + nn],
                                     lhsT=qnT[t6][lo:lo + 64, :R],
                                     rhs=knT[t6][lo:lo + 64, o0:o0 + nn],
                                     start=True, stop=True)
                u = p4.tile([R, N], f32, tag="u")
                nc.vector.tensor_mul(u, raw, rp)
                t2 = p4.tile([R, N], f32, tag="t2")
                nc.scalar.activation(out=t2, in_=mr, func=Act.Identity,
                                     bias=c0_sb[:, h:h + 1],
                                     scale=w1_sb[:, h:h + 1])
                spre = p4.tile([R, N], f32, tag="spre")
                nc.vector.scalar_tensor_tensor(out=spre, in0=qk,
                                               scalar=0.125, in1=u,
                                               op0=Alu.mult, op1=Alu.add)
                if flags["mask"]:
                    nc.vector.tensor_add(spre, spre, cbias_sb[:R])
                s = p4.tile([R, N], f32, tag="s")
                nc.vector.tensor_add(s, spre, t2)
                rowmax = p4.tile([R, 1], f32, tag="rowmax")
                nc.vector.reduce_max(out=rowmax, in_=s,
                                     axis=mybir.AxisListType.X)
                negm = p4.tile([R, 1], f32, tag="negm")
                nc.vector.tensor_scalar_mul(negm, rowmax, -1.0)
                attn = p4.tile([R, N], bf, tag="attn")
                nc.scalar.activation(out=attn, in_=s, func=Act.Exp,
                                     bias=negm, scale=1.0,
                                     accum_out=den[:, h:h + 1])
                for t in range(6):
                    at = p4.tile([128, R], bf, tag="at")
                    _txp(nc, at[:, :R], attn[:, t * 128:(t + 1) * 128])
                    nc.tensor.matmul(ops[:, h * 64:(h + 1) * 64],
                                     lhsT=at[:, :R],
                                     rhs=v_sb[t][:, h * 64:(h + 1) * 64],
                                     start=(t == 0), stop=(t == 5))

            p4ps_cm.__exit__(None, None, None)
            rec = p4s.tile([R, H], f32)
            nc.vector.reciprocal(out=rec, in_=den)
            on = p4s.tile([R, D], bf)
            onv = on.rearrange("p (h f) -> p h f", h=H)
            opsv = ops.rearrange("p (h f) -> p h f", h=H)
            for h in range(H):
                nc.vector.tensor_scalar_mul(onv[:, h, :], opsv[:, h, :],
                                            rec[:, h:h + 1])
            p4po_cm.__exit__(None, None, None)

            # output projection + gate
            p4pz_cm = tc.tile_pool(name="p4pz", bufs=1, space="PSUM")
            p4pz = p4pz_cm.__enter__()
            oproj = p4pz.tile([R, D], f32)
            gate = p4pz.tile([R, D], f32)
            onT = [p4s.tile([128, R], bf, tag=f"onT{t}", name=f"onT{t}") for t in range(6)]
            for t in range(6):
                _txp(nc, onT[t][:, :R], on[:, t * 128:(t + 1) * 128])
            for (o0, nn) in [(0, 512), (512, 256)]:
                for c in range(6):
                    nc.tensor.matmul(oproj[:, o0:o0 + nn], lhsT=onT[c][:, :R],
                                     rhs=Wo_sb[c][:, o0:o0 + nn],
                                     start=(c == 0), stop=(c == 5))
                for c in range(4):
                    nc.tensor.matmul(gate[:, o0:o0 + nn], lhsT=crT[c][:, :R],
                                     rhs=Wsc_sb[c][:, o0:o0 + nn],
                                     start=(c == 0), stop=(c == 3))
            if flags["sbvec"]:
                nc.vector.tensor_add(gate, gate, sb_sb[:R])
                sbias = 0.0
            else:
                sbias = flags["sbconst"]
            sb_c = p4s.tile([R, 1], f32)
            nc.vector.memset(sb_c, float(sbias))
            sig = p4s.tile([R, D], bf)
            nc.scalar.activation(out=sig, in_=gate, func=Act.Sigmoid,
                                 bias=sb_c, scale=1.0)
            fin = p4s.tile([R, D], f32)
            if flags["bo"]:
                nc.vector.tensor_add(fin, oproj, bo_sb[:R])
                nc.vector.tensor_mul(fin, fin, sig)
            else:
                nc.vector.tensor_mul(fin, oproj, sig)
            nc.sync.dma_start(out=out_t[:, :], in_=fin)
            p4pz_cm.__exit__(None, None, None)

        for p in reversed(ctx_pools):
            p.__exit__(None, None, None)

    _split_excess_dma_waits(nc)
    return nc


# --------------------------------------------------------------------------
# host side
# --------------------------------------------------------------------------

_CACHE = {}


def _get_program(flags):
    key = tuple(sorted(flags.items()))
    if key not in _CACHE:
        _CACHE[key] = build_program(flags)
    return _CACHE[key]


def kernel(**inputs):
    x = np.asarray(inputs["x"], F32)
    pair_rep = np.asarray(inputs["pair_rep"], F32)
    cond = np.asarray(inputs["cond"], F32)
    mask = np.asarray(inputs["mask"])
    b = x.shape[0]
    assert b == 1 and x.shape[1] == N

    g = np.asarray(inputs["adaln_cond_gamma"], F32)
    be = np.asarray(inputs["adaln_cond_beta"], F32)
    bg = np.asarray(inputs["adaln_bg"], F32)
    bq = np.asarray(inputs["bq"], F32); bk = np.asarray(inputs["bk"], F32)
    bv = np.asarray(inputs["bv"], F32); bo = np.asarray(inputs["bo"], F32)
    qg = np.asarray(inputs["qln_g"], F32); qb = np.asarray(inputs["qln_b"], F32)
    kg = np.asarray(inputs["kln_g"], F32); kb = np.asarray(inputs["kln_b"], F32)
    pg = np.asarray(inputs["pair_ln_g"], F32)
    pb = np.asarray(inputs["pair_ln_b"], F32)
    Wpb_in = np.asarray(inputs["W_pair_bias"], F32)
    sb = np.asarray(inputs["scale_b"], F32)
    mf = mask.astype(F32)[0]  # [N]

    flags = dict(
        bg=bool(np.any(bg)), bq=bool(np.any(bq)), bk=bool(np.any(bk)),
        bv=bool(np.any(bv)), bo=bool(np.any(bo)),
        qg=not np.all(qg == 1), qb=bool(np.any(qb)),
        kg=not np.all(kg == 1), kb=bool(np.any(kb)),
        cg=not np.all(g == 1), cb=bool(np.any(be)),
        sbvec=bool(np.ptp(sb) != 0),
        sbconst=float(sb[0]) if np.ptp(sb) == 0 else 0.0,
        mask=not np.all(mf == 1),
    )
    nc = _get_program(flags)

    # shared host-prep
    Wgp = (pg[:, None] * Wpb_in).astype(F32)          # [128, 12]
    w1 = Wgp.sum(0)                                   # [12]
    c0 = pb @ Wpb_in                                  # [12]
    Wpb16 = np.zeros((DP, 64), F32)
    Wpb16[:, 0:12] = Wgp
    Wpb16[:, 12] = 1.0 / DP        # mu column (stream a)
    Wpb16[:, 32 + 13] = 1.0 / DP   # E2 column (stream b)
    Wpb16 = Wpb16.astype(BF16)

    com = dict(
        x_all=x[0].astype(BF16), cond_all=cond[0].astype(BF16),
        Wg=np.asarray(inputs["adaln_Wg"], F32).astype(BF16),
        Wb=np.asarray(inputs["adaln_Wb"], F32).astype(BF16),
        Wsc=np.asarray(inputs["scale_W"], F32).astype(BF16),
        Wq=np.asarray(inputs["Wq"], F32).astype(BF16),
        Wk=np.asarray(inputs["Wk"], F32).astype(BF16),
        Wv=np.asarray(inputs["Wv"], F32).astype(BF16),
        Wo=np.asarray(inputs["Wo"], F32).astype(BF16),
        Wpb=Wpb16,
        c0t=np.tile(c0, (R, 1)).astype(F32),
        w1t=np.tile(-w1, (R, 1)).astype(F32),
    )
    def bcast(vec, w):
        return np.tile(vec[None, :], (128, 1)).astype(F32)
    if flags["bg"]:
        com["bg_t"] = bg.reshape(6, 128).T.copy().astype(F32)
    if flags["bq"]:
        com["bq_t"] = bcast(bq, D)
    if flags["bk"]:
        com["bk_t"] = bcast(bk, D)
    if flags["bv"]:
        com["bv_t"] = bcast(bv, D)
    if flags["bo"]:
        com["bo_t"] = bcast(bo, D)
    if flags["qg"]:
        com["qg_t"] = bcast(np.tile(qg, H), D)
    if flags["qb"]:
        com["qb_t"] = bcast(np.tile(qb, H), D)
    if flags["kg"]:
        com["kg_t"] = bcast(np.tile(kg, H), D)
    if flags["kb"]:
        com["kb_t"] = bcast(np.tile(kb, H), D)
    if flags["cg"]:
        com["cg_t"] = bcast(g, DC)
    if flags["cb"]:
        com["cbeta_t"] = bcast(be, DC)
    if flags["sbvec"]:
        com["sb_t"] = bcast(sb, D)
    if flags["mask"]:
        com["cbias_t"] = bcast((mf - 1.0) * 1e9, N)

    pair_b = pair_rep[0].astype(BF16)  # [N, N, DP]
    in_maps = []
    for c in range(NCORES):
        r0 = c * R
        m = dict(com)
        m["pair_bf"] = np.ascontiguousarray(
            pair_b[r0:r0 + R].reshape(NPAIR, DP))
        m["x_rows"] = np.ascontiguousarray(x[0, r0:r0 + R]).astype(BF16)
        m["cond_rows"] = np.ascontiguousarray(cond[0, r0:r0 + R]).astype(BF16)
        in_maps.append(m)

    res = run_bass_kernel_spmd(nc, in_maps, core_ids=list(range(NCORES)))
    out = np.concatenate([np.asarray(res.results[c]["out"], F32)
                          for c in range(NCORES)], axis=0)
    out = out * mf[:, None]
    return out[None]
